# revision 25
# baseline (speedup 1.0000x reference)
"""Trainium2 Bass kernel for the 10-class supervised-contrastive loss.

Problem shapes (hardcoded): preds [10, 2048, 128] f32, target [2048] int64,
log_vars [10] f32 -> scalar f32.

Sharding (8 cores, SPMD, identical program per core):
  - core c owns class c fully (16 row-strips of 128 rows of the [B,B] matrix)
  - cores 0-3 additionally own a quarter of class 8, cores 4-7 a quarter of
    class 9.  The extra class's rows/labels are fed ROTATED (np.roll) so every
    core statically computes row-strips 0..3 of its "slot 1" class; row sums
    are permutation-invariant so rotation is safe (diagonal stays diagonal).

Device, per class (slot 0 exploits exp-matrix symmetry, slot 1 is full rows):
  per 128-row strip a (descending order so short strips warm the pipeline
  while the G DMA streams in tail-first):
      C = G[:,a].T @ G[:, cols]     (bf16 matmuls, f32 PSUM, 512-col chunks)
      sc = Exp(C/T)                 (ACT, bf16 out to SBUF)
      rowsum((sc * maskZ view))    (DVE tensor_tensor_reduce: the diag
          element is zeroed by a shifted view of one [128, 2432] mask with
          zeros along the line j - p == 384; elementwise product goes to a
          scratch tile nobody reads)
      slot 0 only: colsums of computed tiles via PE with a tiny one-hot
      [128,4] stationary and sc as the 512-wide moving operand, accumulated
      across strips into one persistent memset-initialized [4,512] PSUM bank
      ("mirror": row j holds cols 512j..512j+512 of the flat [2048] mirror
      vector).  By symmetry these colsums are the row sums of the skipped
      lower tiles.  Diag-tile cols are excluded (strip a starts at
      (a+1)*128; strip 0's 0:128 region is never written -> memset zeros).

Host prep (O(B*D)): row-normalize features, cast bf16, transpose.
Host epilogue (O(B*D*C)): Z = rowsum + mirror; P/R per-row pos/all cosine
sums from u-vector matmuls in f64; masked mean log-prob with analytic
counts; uncertainty-weighted final sum.
"""

import ml_dtypes
import numpy as np

import concourse.bacc as bacc
import concourse.bass as bass
import concourse.mybir as mybir
import concourse.tile as tile
from concourse.bass_utils import run_bass_kernel_spmd

NUM_CLASSES = 10
B = 2048
D = 128
T = 0.07
BASE_T = 0.07
N_CORES = 8
MZOFF = 384  # stored mask-zero diagonal offset: maskZ[p, j] = 0 iff j-p == MZOFF

f32 = mybir.dt.float32
bf16 = mybir.dt.bfloat16
np_bf16 = ml_dtypes.bfloat16

# (slot, row_strip) units every core executes.  Slot-0 strips mostly
# descend (early strips need only the tail chunks of the g0 DMA stream);
# one slot-1 strip is interleaved after every four.  Strip 15 (narrowest,
# no csum) runs LAST so the closing reduce->DMA chain is short.
UNITS = []
for _i, _a in enumerate(range(14, -1, -1)):
    UNITS.append((0, _a))
    if _i % 4 == 2:
        UNITS.append((1, _i // 4))
UNITS.append((0, 15))

TRACE = False
USE_TTR = False
LAST_RESULT = None


def _chunks512(c0, c1):
    """Split [c0, c1) at 512-aligned boundaries (PSUM bank limit)."""
    out = []
    c = c0
    while c < c1:
        nxt = min(c1, (c // 512 + 1) * 512)
        out.append((c, nxt))
        c = nxt
    return out


def _build_nc():
    nc = bacc.Bacc(None, target_bir_lowering=False)

    g_dram = [
        nc.dram_tensor(f"g{s}", [128, B], bf16, kind="ExternalInput")
        for s in range(2)
    ]
    # All diag-mask windows live in cols [MZOFF-384, MZOFF+128) = [0, 512).
    maskz_dram = nc.dram_tensor("maskz", [128, 512], bf16, kind="ExternalInput")
    # cols 0..16: four one-hot [128,4] blocks; cols 16..20: zeros (used as
    # zero weights to open the mirror PSUM accumulation group).
    ones16_dram = nc.dram_tensor("ones16", [128, 20], bf16, kind="ExternalInput")
    out_dram = nc.dram_tensor("out", [128, 20], f32, kind="ExternalOutput")
    mirror_dram = nc.dram_tensor("mirror", [4, 512], f32, kind="ExternalOutput")

    EXP = mybir.ActivationFunctionType.Exp
    add = mybir.AluOpType.add
    mult = mybir.AluOpType.mult

    with tile.TileContext(nc) as tc:
        with (
            tc.tile_pool(name="const", bufs=1) as constp,
            tc.tile_pool(name="gmat", bufs=1) as gmatp,
            tc.tile_pool(name="scp", bufs=4) as scp,
        ):
            ones16_sb = constp.tile([128, 20], bf16, tag="ones16")
            nc.scalar.dma_start(ones16_sb[:], ones16_dram[:])
            maskz_sb = constp.tile([128, 512], bf16, tag="maskz")
            nc.scalar.dma_start(maskz_sb[:], maskz_dram[:])
            out_sb = constp.tile([128, 20], f32, tag="out")
            mirror_sb = constp.tile([4, 512], f32, tag="mirror_sb")

            # G streams in tail-first so descending strips can start early.
            # All G transfers ride the otherwise-idle sync queue; the scalar
            # queue stays free for the ACT table load + activations.
            G = []
            for s in range(2):
                g = gmatp.tile([128, B], bf16, tag=f"G{s}", name=f"G{s}")
                for q0, q1 in ((1536, 2048), (1024, 1536), (512, 1024), (0, 512)):
                    nc.sync.dma_start(g[:, q0:q1], g_dram[s][:, q0:q1])
                G.append(g)

            with (
                tc.tile_pool(name="cpsum", bufs=2, space="PSUM") as cpp,
                tc.tile_pool(name="mirpsum", bufs=1, space="PSUM") as mirp,
            ):
                # mirror[j, c] accumulates colsums for flat col 512*j + c.
                # Zero-weight matmul opens the accumulation group (writes
                # exact zeros) so later csums can accumulate in any order.
                mirror = mirp.tile([4, 512], f32, tag="mirror", name="mirror")
                nc.tensor.matmul(
                    mirror[:],
                    ones16_sb[:, 16:20],
                    maskz_sb[:, 0:512],
                    start=True,
                    stop=False,
                    skip_group_check=True,
                )

                for u, (s, a) in enumerate(UNITS):
                    c0 = 128 * a if s == 0 else 0
                    lhsT = G[s][:, 128 * a : 128 * a + 128]
                    sc = scp.tile([128, B], bf16, tag="sc", name=f"sc{u}")

                    # 3-bank [128,1536] PSUM tiles: strips starting at col
                    # >= 512 take a single activation over (c0, 2048).
                    halves = [(c0, 1024), (1024, 2048)] if c0 < 512 else [(c0, 2048)]
                    for h0, h1 in halves:
                        base = h1 - 1536 if h1 - h0 > 1024 else h1 - 1024
                        cp = cpp.tile([128, 1536], f32, tag="cp", name=f"cp{u}_{h0}")
                        for s0, s1 in _chunks512(h0, h1):
                            nc.tensor.matmul(
                                cp[:, s0 - base : s1 - base],
                                lhsT,
                                G[s][:, s0:s1],
                                start=True,
                                stop=True,
                            )
                        nc.scalar.activation(
                            sc[:, h0:h1],
                            cp[:, h0 - base : h1 - base],
                            EXP,
                            scale=1.0 / T,
                        )

                    # Diag-zeroing (the [MZOFF:MZOFF+128] mask window is
                    # (1-I)) on idle GPSIMD, then row sum on DVE.
                    d0 = 128 * a
                    nc.gpsimd.tensor_mul(
                        sc[:, d0 : d0 + 128],
                        sc[:, d0 : d0 + 128],
                        maskz_sb[:, MZOFF : MZOFF + 128],
                    )
                    nc.vector.tensor_reduce(
                        out_sb[:, u : u + 1],
                        sc[:, c0:B],
                        axis=mybir.AxisListType.X,
                        op=add,
                    )

                    if s == 0 and a < 15:
                        # Colsums of computed tiles = rowsums of mirrored
                        # skipped tiles (diag-tile cols excluded).
                        for s0, s1 in _chunks512(c0 + 128, 2048):
                            j = s0 // 512
                            nc.tensor.matmul(
                                mirror[:, s0 - 512 * j : s1 - 512 * j],
                                ones16_sb[:, 4 * j : 4 * j + 4],
                                sc[:, s0:s1],
                                start=False,
                                stop=(a == 0),
                                skip_group_check=True,
                            )

                nc.scalar.copy(mirror_sb[:], mirror[:])

            nc.sync.dma_start(out_dram[:], out_sb[:])
            nc.sync.dma_start(mirror_dram[:], mirror_sb[:])
    nc.finalize()
    return nc


_NC_CACHE = None


def _get_nc():
    global _NC_CACHE
    if _NC_CACHE is None:
        _NC_CACHE = _build_nc()
    return _NC_CACHE


def kernel(preds, target, log_vars):
    global LAST_RESULT
    preds = np.asarray(preds, dtype=np.float32)
    target = np.asarray(target)
    log_vars = np.asarray(log_vars, dtype=np.float32)

    onehot = (target[None, :] == np.arange(NUM_CLASSES, dtype=target.dtype)[:, None])
    onehot = onehot.astype(np.float32)  # [10, B]
    npos = onehot.sum(axis=1).astype(np.float64)  # [10]

    # Host prep: row-normalize (f32 stats), cast bf16 for the device.
    norms = np.sqrt((preds**2).sum(axis=2, dtype=np.float32))
    ghat_f = preds / norms[:, :, None]  # [10, B, D] f32
    ghat = ghat_f.astype(np_bf16)

    jj = np.arange(512)[None, :]
    pp = np.arange(128)[:, None]
    maskz = (jj - pp != MZOFF).astype(np_bf16)  # [128, 512]; (1-I) at [*,384:512]
    ones16 = np.zeros((128, 20), dtype=np_bf16)
    for j in range(4):
        ones16[:, 4 * j + j] = 1.0

    in_maps = []
    for c in range(N_CORES):
        cls1 = 8 + c // 4
        off = 512 * (c % 4)
        im = {"maskz": maskz, "ones16": ones16}
        for s, (cls, o) in enumerate([(c, 0), (cls1, off)]):
            gh = np.roll(ghat[cls], -o, axis=0) if o else ghat[cls]
            im[f"g{s}"] = np.ascontiguousarray(gh.T)  # [128, 2048] [d, b]
        in_maps.append(im)

    nc = _get_nc()
    res = run_bass_kernel_spmd(nc, in_maps, list(range(N_CORES)), trace=TRACE)
    LAST_RESULT = res

    # Reassemble per-(class,row) Z (sum over off-diag exp logits).
    Z = np.zeros((NUM_CLASSES, B), dtype=np.float64)
    rows128 = np.arange(128)
    for c in range(N_CORES):
        o = np.asarray(res.results[c]["out"], dtype=np.float64)  # [128, 20]
        mir = np.asarray(res.results[c]["mirror"], dtype=np.float64).reshape(-1)
        mir[0:128] = 0.0
        for u, (s, a) in enumerate(UNITS):
            if s == 0:
                rows = 128 * a + rows128
                Z[c, rows] = o[:, u] + mir[rows]
            else:
                cls, base = 8 + c // 4, 512 * (c % 4)
                rows = (base + 128 * a + rows128) % B
                Z[cls, rows] = o[:, u]

    # P/R per-row positive/total cosine sums (O(B*D*C), f64 on host).
    g64 = ghat_f.astype(np.float64)
    lab = onehot.astype(np.float64)
    u_all = g64.sum(axis=1)  # [10, D]
    u_pos = np.einsum("cbd,cb->cd", g64, lab)  # [10, D]
    P = np.einsum("cbd,cd->cb", g64, u_pos)  # [10, B]
    R = np.einsum("cbd,cd->cb", g64, u_all)  # [10, B]

    masked_cos = lab * P + (1.0 - lab) * (R - P)
    masked_logits_sum = (masked_cos - 1.0) / T
    cnt = lab * npos[:, None] + (1.0 - lab) * (B - npos[:, None]) - 1.0
    mlpp = masked_logits_sum / cnt - np.log(Z)
    losses = -(T / BASE_T) * mlpp.mean(axis=1)  # [10]
    lv = log_vars.astype(np.float64)
    final = np.sum(np.exp(-lv) * losses + lv)
    return np.float32(final)


# revision 26
# speedup vs baseline: 1.1281x; 1.1281x over previous
"""Trainium2 Bass kernel for the 10-class supervised-contrastive loss.

Problem shapes (hardcoded): preds [10, 2048, 128] f32, target [2048] int64,
log_vars [10] f32 -> scalar f32.

Sharding (8 cores, SPMD, identical program per core):
  - core c owns class c fully (16 row-strips of 128 rows of the [B,B] matrix)
  - cores 0-3 additionally own a quarter of class 8, cores 4-7 a quarter of
    class 9.  The extra class's rows/labels are fed ROTATED (np.roll) so every
    core statically computes row-strips 0..3 of its "slot 1" class; row sums
    are permutation-invariant so rotation is safe (diagonal stays diagonal).

Device, per class (slot 0 exploits exp-matrix symmetry, slot 1 is full rows):
  per 128-row strip a (descending order so short strips warm the pipeline
  while the G DMA streams in tail-first):
      C = G[:,a].T @ G[:, cols]     (bf16 matmuls, f32 PSUM, 512-col chunks)
      sc = Exp(C/T)                 (ACT, bf16 out to SBUF)
      rowsum((sc * maskZ view))    (DVE tensor_tensor_reduce: the diag
          element is zeroed by a shifted view of one [128, 2432] mask with
          zeros along the line j - p == 384; elementwise product goes to a
          scratch tile nobody reads)
      slot 0 only: colsums of computed tiles via PE with a tiny one-hot
      [128,4] stationary and sc as the 512-wide moving operand, accumulated
      across strips into one persistent memset-initialized [4,512] PSUM bank
      ("mirror": row j holds cols 512j..512j+512 of the flat [2048] mirror
      vector).  By symmetry these colsums are the row sums of the skipped
      lower tiles.  Diag-tile cols are excluded (strip a starts at
      (a+1)*128; strip 0's 0:128 region is never written -> memset zeros).

Host prep (O(B*D)): row-normalize features, cast bf16, transpose.
Host epilogue (O(B*D*C)): Z = rowsum + mirror; P/R per-row pos/all cosine
sums from u-vector matmuls in f64; masked mean log-prob with analytic
counts; uncertainty-weighted final sum.
"""

import ml_dtypes
import numpy as np

import concourse.bacc as bacc
import concourse.bass as bass
import concourse.mybir as mybir
import concourse.tile as tile
from concourse.bass_utils import run_bass_kernel_spmd

NUM_CLASSES = 10
B = 2048
D = 128
T = 0.07
BASE_T = 0.07
N_CORES = 8
MZOFF = 384  # stored mask-zero diagonal offset: maskZ[p, j] = 0 iff j-p == MZOFF

f32 = mybir.dt.float32
bf16 = mybir.dt.bfloat16
np_bf16 = ml_dtypes.bfloat16

# (slot, row_strip) units every core executes.  Slot-0 strips mostly
# descend (early strips need only the tail chunks of the g0 DMA stream);
# one slot-1 strip is interleaved after every four.  Strip 15 (narrowest,
# no csum) runs LAST so the closing reduce->DMA chain is short.
UNITS = []
for _i, _a in enumerate(range(14, -1, -1)):
    UNITS.append((0, _a))
    if _i % 4 == 2:
        UNITS.append((1, _i // 4))
UNITS.append((0, 15))

TRACE = False
USE_TTR = False
LAST_RESULT = None


def _chunks512(c0, c1):
    """Split [c0, c1) at 512-aligned boundaries (PSUM bank limit)."""
    out = []
    c = c0
    while c < c1:
        nxt = min(c1, (c // 512 + 1) * 512)
        out.append((c, nxt))
        c = nxt
    return out


def _build_nc():
    nc = bacc.Bacc(None, target_bir_lowering=False)

    g_dram = [
        nc.dram_tensor(f"g{s}", [128, B], bf16, kind="ExternalInput")
        for s in range(2)
    ]
    # All diag-mask windows live in cols [MZOFF-384, MZOFF+128) = [0, 512).
    maskz_dram = nc.dram_tensor("maskz", [128, 512], bf16, kind="ExternalInput")
    # cols 0..16: four one-hot [128,4] blocks; cols 16..20: zeros (used as
    # zero weights to open the mirror PSUM accumulation group).
    ones16_dram = nc.dram_tensor("ones16", [128, 20], bf16, kind="ExternalInput")
    out_dram = nc.dram_tensor("out", [128, 20], f32, kind="ExternalOutput")
    mirror_dram = nc.dram_tensor("mirror", [4, 512], f32, kind="ExternalOutput")

    EXP = mybir.ActivationFunctionType.Exp
    add = mybir.AluOpType.add
    mult = mybir.AluOpType.mult

    with tile.TileContext(nc) as tc:
        with (
            tc.tile_pool(name="const", bufs=1) as constp,
            tc.tile_pool(name="gmat", bufs=1) as gmatp,
            tc.tile_pool(name="scp", bufs=4) as scp,
        ):
            ones16_sb = constp.tile([128, 20], bf16, tag="ones16")
            nc.scalar.dma_start(ones16_sb[:], ones16_dram[:])
            maskz_sb = constp.tile([128, 512], bf16, tag="maskz")
            nc.scalar.dma_start(maskz_sb[:], maskz_dram[:])
            out_sb = constp.tile([128, 20], f32, tag="out")
            mirror_sb = constp.tile([4, 512], f32, tag="mirror_sb")

            # G streams in tail-first so descending strips can start early.
            # All G transfers ride the otherwise-idle sync queue; the scalar
            # queue stays free for the ACT table load + activations.
            G = []
            for s, eng in ((0, nc.scalar), (1, nc.gpsimd)):
                g = gmatp.tile([128, B], bf16, tag=f"G{s}", name=f"G{s}")
                for q0, q1 in ((1536, 2048), (1024, 1536), (512, 1024), (0, 512)):
                    eng.dma_start(g[:, q0:q1], g_dram[s][:, q0:q1])
                G.append(g)

            with (
                tc.tile_pool(name="cpsum", bufs=2, space="PSUM") as cpp,
                tc.tile_pool(name="mirpsum", bufs=1, space="PSUM") as mirp,
            ):
                # mirror[j, c] accumulates colsums for flat col 512*j + c.
                # Zero-weight matmul opens the accumulation group (writes
                # exact zeros) so later csums can accumulate in any order.
                mirror = mirp.tile([4, 512], f32, tag="mirror", name="mirror")
                nc.tensor.matmul(
                    mirror[:],
                    ones16_sb[:, 16:20],
                    maskz_sb[:, 0:512],
                    start=True,
                    stop=False,
                    skip_group_check=True,
                )

                for u, (s, a) in enumerate(UNITS):
                    c0 = 128 * a if s == 0 else 0
                    lhsT = G[s][:, 128 * a : 128 * a + 128]
                    sc = scp.tile([128, B], bf16, tag="sc", name=f"sc{u}")

                    # 3-bank [128,1536] PSUM tiles: strips starting at col
                    # >= 512 take a single activation over (c0, 2048).
                    halves = [(c0, 1024), (1024, 2048)] if c0 < 512 else [(c0, 2048)]
                    for h0, h1 in halves:
                        base = h1 - 1536 if h1 - h0 > 1024 else h1 - 1024
                        cp = cpp.tile([128, 1536], f32, tag="cp", name=f"cp{u}_{h0}")
                        for s0, s1 in _chunks512(h0, h1):
                            nc.tensor.matmul(
                                cp[:, s0 - base : s1 - base],
                                lhsT,
                                G[s][:, s0:s1],
                                start=True,
                                stop=True,
                            )
                        nc.scalar.activation(
                            sc[:, h0:h1],
                            cp[:, h0 - base : h1 - base],
                            EXP,
                            scale=1.0 / T,
                        )

                    # Diag-zeroing (the [MZOFF:MZOFF+128] mask window is
                    # (1-I)) on idle GPSIMD, then row sum on DVE.
                    d0 = 128 * a
                    nc.gpsimd.tensor_mul(
                        sc[:, d0 : d0 + 128],
                        sc[:, d0 : d0 + 128],
                        maskz_sb[:, MZOFF : MZOFF + 128],
                    )
                    nc.vector.tensor_reduce(
                        out_sb[:, u : u + 1],
                        sc[:, c0:B],
                        axis=mybir.AxisListType.X,
                        op=add,
                    )

                    if s == 0 and a < 15:
                        # Colsums of computed tiles = rowsums of mirrored
                        # skipped tiles (diag-tile cols excluded).
                        for s0, s1 in _chunks512(c0 + 128, 2048):
                            j = s0 // 512
                            nc.tensor.matmul(
                                mirror[:, s0 - 512 * j : s1 - 512 * j],
                                ones16_sb[:, 4 * j : 4 * j + 4],
                                sc[:, s0:s1],
                                start=False,
                                stop=(a == 0),
                                skip_group_check=True,
                            )

                nc.scalar.copy(mirror_sb[:], mirror[:])

            nc.sync.dma_start(out_dram[:], out_sb[:])
            nc.sync.dma_start(mirror_dram[:], mirror_sb[:])
    nc.finalize()
    return nc


_NC_CACHE = None


def _get_nc():
    global _NC_CACHE
    if _NC_CACHE is None:
        _NC_CACHE = _build_nc()
    return _NC_CACHE


def kernel(preds, target, log_vars):
    global LAST_RESULT
    preds = np.asarray(preds, dtype=np.float32)
    target = np.asarray(target)
    log_vars = np.asarray(log_vars, dtype=np.float32)

    onehot = (target[None, :] == np.arange(NUM_CLASSES, dtype=target.dtype)[:, None])
    onehot = onehot.astype(np.float32)  # [10, B]
    npos = onehot.sum(axis=1).astype(np.float64)  # [10]

    # Host prep: row-normalize (f32 stats), cast bf16 for the device.
    norms = np.sqrt((preds**2).sum(axis=2, dtype=np.float32))
    ghat_f = preds / norms[:, :, None]  # [10, B, D] f32
    ghat = ghat_f.astype(np_bf16)

    jj = np.arange(512)[None, :]
    pp = np.arange(128)[:, None]
    maskz = (jj - pp != MZOFF).astype(np_bf16)  # [128, 512]; (1-I) at [*,384:512]
    ones16 = np.zeros((128, 20), dtype=np_bf16)
    for j in range(4):
        ones16[:, 4 * j + j] = 1.0

    in_maps = []
    for c in range(N_CORES):
        cls1 = 8 + c // 4
        off = 512 * (c % 4)
        im = {"maskz": maskz, "ones16": ones16}
        for s, (cls, o) in enumerate([(c, 0), (cls1, off)]):
            gh = np.roll(ghat[cls], -o, axis=0) if o else ghat[cls]
            im[f"g{s}"] = np.ascontiguousarray(gh.T)  # [128, 2048] [d, b]
        in_maps.append(im)

    nc = _get_nc()
    res = run_bass_kernel_spmd(nc, in_maps, list(range(N_CORES)), trace=TRACE)
    LAST_RESULT = res

    # Reassemble per-(class,row) Z (sum over off-diag exp logits).
    Z = np.zeros((NUM_CLASSES, B), dtype=np.float64)
    rows128 = np.arange(128)
    for c in range(N_CORES):
        o = np.asarray(res.results[c]["out"], dtype=np.float64)  # [128, 20]
        mir = np.asarray(res.results[c]["mirror"], dtype=np.float64).reshape(-1)
        mir[0:128] = 0.0
        for u, (s, a) in enumerate(UNITS):
            if s == 0:
                rows = 128 * a + rows128
                Z[c, rows] = o[:, u] + mir[rows]
            else:
                cls, base = 8 + c // 4, 512 * (c % 4)
                rows = (base + 128 * a + rows128) % B
                Z[cls, rows] = o[:, u]

    # P/R per-row positive/total cosine sums (O(B*D*C), f64 on host).
    g64 = ghat_f.astype(np.float64)
    lab = onehot.astype(np.float64)
    u_all = g64.sum(axis=1)  # [10, D]
    u_pos = np.einsum("cbd,cb->cd", g64, lab)  # [10, D]
    P = np.einsum("cbd,cd->cb", g64, u_pos)  # [10, B]
    R = np.einsum("cbd,cd->cb", g64, u_all)  # [10, B]

    masked_cos = lab * P + (1.0 - lab) * (R - P)
    masked_logits_sum = (masked_cos - 1.0) / T
    cnt = lab * npos[:, None] + (1.0 - lab) * (B - npos[:, None]) - 1.0
    mlpp = masked_logits_sum / cnt - np.log(Z)
    losses = -(T / BASE_T) * mlpp.mean(axis=1)  # [10]
    lv = log_vars.astype(np.float64)
    final = np.sum(np.exp(-lv) * losses + lv)
    return np.float32(final)
